# revision 1
# baseline (speedup 1.0000x reference)
"""Trainium2 Bass kernel for nn_MultiHeadAttention_44281112822190.

8 NeuronCores, pure data parallelism over the 8192 (b,s) rows: core c takes
rows [c*1024, (c+1)*1024) (batch b = c//2, s-offset (c%2)*1024). No
collectives; the host shards inputs and reassembles the output.

Math notes:
  - The reference applies RoPE to q and k, then contracts q.k at the SAME
    position (per-position head-head attention [B,S,H,H]). RoPE is an
    orthogonal per-position rotation applied identically to q and k, so it
    cancels exactly in the scores: (R q).(R k) = q.k. The kernel skips RoPE
    entirely (freqs inputs are unused). Validated vs reference at 1.7e-6 in
    fp32 (check_math.py).
  - The reference's "h-major flatten" transpose(0,2,1,3).reshape(B,S,-1) is a
    scramble: out[b, h*128 + s//16, (s%16)*128 + d] = att_out[b, s, h, d].
    Each scrambled row draws from 16 consecutive positions of one head, all
    inside one core's shard, so the output projection stays core-local.

Numerics: all matmul operands are fp16 (10-bit mantissa, like tf32) with
fp32 PSUM accumulation -> ~7e-4 relative error end-to-end, 1 cycle/row on
the PE (4x faster than fp32 and half the HBM traffic).

Per-core structure (one 1024-position block):
  1. Q^T/K^T/V^T projections: stationary = host-transposed weight tiles
     (streamed once, 0.5 MB column-pair chunks), moving = host-transposed x
     (split in 4 tiles so compute starts early); each LDWEIGHTS feeds two
     N=512 matmuls. Layout [128 d, 1024 s, 16 h] (h-minor) makes attention
     group slabs contiguous. Bias added during PSUM->SBUF copyback
     (tensor_scalar_add, per-partition bias).
  2. Attention, 8 positions/group, 2 groups/pair: one 128x128 matmul gives
     scores [(i,h) x (j,g)]; exp(scale*scores) on ScalarE straight from
     PSUM; fused mask-multiply + row-sum (scalar_tensor_tensor accum_out);
     reciprocal; normalize (GpSimd); transpose att and V-slab on TensorE;
     attO^T slab = vT.T @ attT -> [d, (i,h)]; scatter into attO quarters
     [128 d, 16 s_lo, 256 (u*16+h)] - u-major columns make the output
     projection moving operands contiguous (RHS APs must be depth-1).
  3. Output projection from attO quarters with host-transposed wo. Quarter
     k is emitted interleaved with attention quarter k+1, so its matmuls
     fill TensorE while softmax work occupies Vector/Scalar/GpSimd.
Host reassembles the scrambled rows into the final [4, 2048, 2048] output.

Measured on trn2 (8 cores): 584 us HW exec, rel err 6.7e-4.
"""

import os
import sys

sys.path.insert(0, "/opt/trn_rl_repo")

import numpy as np

import concourse.bacc as bacc
import concourse.mybir as mybir
import concourse.tile as tile
from concourse.bass_utils import run_bass_kernel_spmd

F32 = mybir.dt.float32
F16 = mybir.dt.float16
AF = mybir.ActivationFunctionType
ALU = mybir.AluOpType

B, S, E, H, D = 4, 2048, 2048, 16, 128
NCORES = 8
SCALE = 1.0 / float(np.sqrt(D))

_CACHE = {}
LAST_EXEC_NS = None


def _build():
    nc = bacc.Bacc(trn_type="TRN2", target_bir_lowering=False)

    xt = nc.dram_tensor("xt", [16, 128, 1024], F16, kind="ExternalInput")
    wqt = nc.dram_tensor("wqt", [E, E], F16, kind="ExternalInput")
    wkt = nc.dram_tensor("wkt", [E, E], F16, kind="ExternalInput")
    wvt = nc.dram_tensor("wvt", [E, E], F16, kind="ExternalInput")
    wot = nc.dram_tensor("wot", [E, E], F16, kind="ExternalInput")
    bqt = nc.dram_tensor("bqt", [128, 16], F32, kind="ExternalInput")
    bkt = nc.dram_tensor("bkt", [128, 16], F32, kind="ExternalInput")
    bvt = nc.dram_tensor("bvt", [128, 16], F32, kind="ExternalInput")
    bot = nc.dram_tensor("bot", [128, 16], F32, kind="ExternalInput")
    mask01 = nc.dram_tensor("mask01", [128, 128], F32, kind="ExternalInput")
    ident = nc.dram_tensor("ident", [128, 128], F16, kind="ExternalInput")
    out = nc.dram_tensor("out", [16, 128, 1024], F32, kind="ExternalOutput")

    with tile.TileContext(nc) as tc:
        with (
            tc.tile_pool(name="const", bufs=1) as cp,
            tc.tile_pool(name="xp", bufs=1) as xp,
            tc.tile_pool(name="qkv", bufs=1) as qkvp,
            tc.tile_pool(name="aop", bufs=1) as aop,
            tc.tile_pool(name="wp", bufs=3) as wp,
            tc.tile_pool(name="gp", bufs=5) as gp,
            tc.tile_pool(name="op", bufs=3) as op,
            tc.tile_pool(name="pp", bufs=4, space="PSUM") as pp,
            tc.tile_pool(name="pa", bufs=3, space="PSUM") as pa,
            tc.tile_pool(name="pb", bufs=1, space="PSUM") as pb,
        ):
            mask_sb = cp.tile([128, 128], F32, tag="mask")
            id_sb = cp.tile([128, 128], F16, tag="id")
            nc.sync.dma_start(mask_sb[:], mask01[:, :])
            nc.sync.dma_start(id_sb[:], ident[:, :])
            bias_sb = {}
            for name, t_ in (("bq", bqt), ("bk", bkt), ("bv", bvt), ("bo", bot)):
                b_sb = cp.tile([128, 16], F32, tag=name)
                nc.sync.dma_start(b_sb[:], t_[:, :])
                bias_sb[name] = b_sb

            xtb_c = []
            for kc in range(4):
                xc = xp.tile([128, 4, 1024], F16, tag=f"xtb{kc}", name=f"xtb{kc}")
                nc.sync.dma_start(
                    xc[:],
                    xt[4 * kc : 4 * kc + 4, :, :].rearrange("k p s -> p k s"),
                )
                xtb_c.append(xc)

            # --- Q/K/V projections -> [128 d, 1024 s, 16 h] fp16 ---
            qb = qkvp.tile([128, 1024, 16], F16, tag="qb")
            kb = qkvp.tile([128, 1024, 16], F16, tag="kb")
            vb = qkvp.tile([128, 1024, 16], F16, tag="vb")
            for wdram, bias, dst in (
                (wqt, "bq", qb),
                (wkt, "bk", kb),
                (wvt, "bv", vb),
            ):
                for t2 in range(8):
                    wtile = wp.tile([128, 16, 256], F16, tag="w")
                    nc.sync.dma_start(
                        wtile[:],
                        wdram[:, t2 * 256 : (t2 + 1) * 256].rearrange(
                            "(k p) c -> p k c", p=128
                        ),
                    )
                    for half in range(2):
                        t = 2 * t2 + half
                        psA = pp.tile([128, 512], F32, tag="pp")
                        psB = pp.tile([128, 512], F32, tag="pp")
                        for k in range(16):
                            w_ap = wtile[:, k, half * 128 : half * 128 + 128]
                            nc.tensor.matmul(
                                psA[:], w_ap, xtb_c[k // 4][:, k % 4, 0:512],
                                start=(k == 0), stop=(k == 15),
                            )
                            nc.tensor.matmul(
                                psB[:], w_ap, xtb_c[k // 4][:, k % 4, 512:1024],
                                start=(k == 0), stop=(k == 15),
                            )
                        nc.vector.tensor_scalar_add(
                            dst[:, 0:512, t], psA[:], bias_sb[bias][:, t : t + 1]
                        )
                        nc.vector.tensor_scalar_add(
                            dst[:, 512:1024, t], psB[:], bias_sb[bias][:, t : t + 1]
                        )

            # --- attention (pairs) + overlapped output projection halves ---
            # attO half-tiles: [128 d, 16 sl, 512] with col = u_local*16 + h
            attO_h = [
                aop.tile([128, 16, 256], F16, tag=f"attO{q}", name=f"attO{q}")
                for q in range(4)
            ]
            mask_b = mask_sb[:].unsqueeze(1).to_broadcast([128, 2, 128])

            def attn_pair(P2):
                G = 2 * P2
                ga = pa.tile([128, 512], F32, tag="ga")
                tr = pb.tile([128, 512], F16, tag="tr")
                for j in range(2):
                    s0 = (G + j) * 8
                    nc.tensor.matmul(
                        ga[:, 128 * j : 128 * j + 128],
                        qb[:, s0 : s0 + 8, :],
                        kb[:, s0 : s0 + 8, :],
                        start=True, stop=True,
                    )
                e2 = gp.tile([128, 256], F32, tag="e2")
                nc.scalar.activation(e2[:], ga[:, 0:256], AF.Exp, scale=SCALE)
                em2 = e2[:].rearrange("p (g c) -> p g c", g=2)
                den2 = gp.tile([128, 2], F32, tag="den2")
                for j in range(2):
                    nc.vector.scalar_tensor_tensor(
                        em2[:, j, :], e2[:, 128 * j : 128 * j + 128], 1.0,
                        mask_sb[:], ALU.bypass, ALU.mult,
                        accum_out=den2[:, j : j + 1],
                    )
                rec2 = gp.tile([128, 2], F32, tag="rec2")
                nc.vector.reciprocal(rec2[:], den2[:])
                att2 = gp.tile([128, 2, 128], F16, tag="att2")
                nc.gpsimd.tensor_tensor(
                    att2[:], em2, rec2[:].unsqueeze(2).to_broadcast([128, 2, 128]),
                    ALU.mult,
                )
                for j in range(2):
                    s0 = (G + j) * 8
                    nc.tensor.transpose(
                        tr[:, 128 * j : 128 * j + 128], att2[:, j, :], id_sb[:]
                    )
                    nc.tensor.transpose(
                        tr[:, 256 + 128 * j : 384 + 128 * j], vb[:, s0 : s0 + 8, :],
                        id_sb[:],
                    )
                trsb = gp.tile([128, 512], F16, tag="trsb")
                nc.scalar.activation(trsb[:], tr[:], AF.Copy)
                for j in range(2):
                    nc.tensor.matmul(
                        ga[:, 256 + 128 * j : 384 + 128 * j],
                        trsb[:, 256 + 128 * j : 384 + 128 * j],
                        trsb[:, 128 * j : 128 * j + 128],
                        start=True, stop=True,
                    )
                # scatter: psum cols (g2, i, h) -> attO_h[u_hi][:, (g2,i), u_lo*16+h]
                u_hi, u_lo = P2 // 16, P2 % 16
                dst = attO_h[u_hi][:].rearrange(
                    "p (g2 i) (u h) -> p g2 i u h", g2=2, h=16
                )[:, :, :, u_lo, :]
                nc.vector.tensor_copy(dst, ga[:, 256:512])

            def final_w_dma(t2):
                wtile = wp.tile([128, 16, 256], F16, tag="w")
                nc.sync.dma_start(
                    wtile[:],
                    wot[:, t2 * 256 : (t2 + 1) * 256].rearrange(
                        "(k p) c -> p k c", p=128
                    ),
                )
                return wtile

            def final_t(q, t2, half, wtile):
                t = 2 * t2 + half
                ps = pp.tile([128, 512], F32, tag="pp")
                for sl in range(16):
                    nc.tensor.matmul(
                        ps[:, 0:256],
                        wtile[:, sl, half * 128 : half * 128 + 128],
                        attO_h[q][:, sl, :],
                        start=(sl == 0), stop=(sl == 15),
                    )
                ob = op.tile([128, 512], F32, tag="ob")
                nc.vector.tensor_scalar_add(
                    ob[:, 0:256], ps[:, 0:256], bias_sb["bo"][:, t : t + 1]
                )
                nc.sync.dma_start(
                    out[t, :, q * 256 : q * 256 + 256], ob[:, 0:256]
                )

            for P2 in range(16):
                attn_pair(P2)
            for q in range(3):
                # interleave final-quarter q (16 t-chunks) 1:1 with the 16
                # attention pairs of quarter q+1
                for i in range(8):
                    wtile = final_w_dma(i)
                    for half in range(2):
                        final_t(q, i, half, wtile)
                        attn_pair(16 * (q + 1) + 2 * i + half)
            for t2 in range(8):
                wtile = final_w_dma(t2)
                for half in range(2):
                    final_t(3, t2, half, wtile)

    nc.compile()
    return nc


def _get_nc():
    if "nc" not in _CACHE:
        _CACHE["nc"] = _build()
    return _CACHE["nc"]


def make_in_maps(inputs):
    x = np.ascontiguousarray(np.asarray(inputs["x"], dtype=np.float32))
    ws = {k: np.asarray(inputs[k], dtype=np.float32) for k in ("wq", "wk", "wv", "wo")}
    bs = {k: np.asarray(inputs[k], dtype=np.float32) for k in ("bq", "bk", "bv", "bo")}

    xf = x.reshape(B * S, E)
    f16 = lambda a: np.ascontiguousarray(a).astype(np.float16)
    btile = lambda b: np.ascontiguousarray(b.reshape(16, 128).T)
    ii = np.arange(128) // 16
    mask01 = (ii[:, None] == ii[None, :]).astype(np.float32)
    common = {
        "wqt": f16(ws["wq"].T), "wkt": f16(ws["wk"].T),
        "wvt": f16(ws["wv"].T), "wot": f16(ws["wo"].T),
        "bqt": btile(bs["bq"]), "bkt": btile(bs["bk"]),
        "bvt": btile(bs["bv"]), "bot": btile(bs["bo"]),
        "mask01": mask01, "ident": np.eye(128, dtype=np.float16),
    }
    in_maps = []
    for c in range(NCORES):
        xt_c = f16(xf[c * 1024 : (c + 1) * 1024].T).reshape(16, 128, 1024)
        in_maps.append({"xt": xt_c, **common})
    return in_maps


def assemble(results):
    out = np.empty((B, S, E), np.float32)
    for c in range(NCORES):
        O = results[c]["out"]  # [16 t, 128 p, 1024]; col = u*16 + h
        Oc = O.reshape(E, 64, 16)  # [j, u, h]
        tgt = out[c // 2].reshape(16, 128, E)
        v0 = (c % 2) * 64
        tgt[:, v0 : v0 + 64, :] = Oc.transpose(2, 1, 0)
    return out


def kernel(**inputs):
    global LAST_EXEC_NS
    nc = _get_nc()
    res = run_bass_kernel_spmd(nc, make_in_maps(inputs), core_ids=list(range(NCORES)))
    LAST_EXEC_NS = res.exec_time_ns
    return assemble(res.results)

